# revision 35
# baseline (speedup 1.0000x reference)
"""Trainium2 Bass kernel: CausalParticleAttention (time-major, causal-skipped).

Problem: B=16 batches, N=16 particles, T=48 timesteps, C=512 channels,
H=8 heads (hd=64), attention over L=N*T=768 with per-head relative
time-position bias (T x T), relative particle-position bias (N x N) and a
causal mask over the time axis only; then output projection.

Sharding: pure data parallel over batch, 2 batches per NeuronCore x 8 cores.

Key layout choice: rows are TIME-MAJOR (r = t*NP + n). The causal mask
(t2 > t1 masked, particles all-visible) then makes S block-lower-triangular
at 128-row granularity (128 rows = 8 timesteps x 16 particles), so QK^T,
exp and PV skip the strictly-upper chunks (~1/3 of attention work).

Per-core algorithm (all matmuls in float32r):
  1. x^T is staged pre-transposed on the HOST (xt input) - no PE transposes.
  2. q,k projections produce [128 c_out, L] PSUM tiles; the two 64-dim head
     halves are copied straight into persistent per-(b,h) augmented tiles:
       qa[(b,h)] = [q_h (64 rows) | onehot(t1,n1) (64 rows)]   (i-indexed)
       ka[(b,h)] = [k_h (64 rows) | 8*bias_t+mask | 8*bias_p]  (j-indexed)
     so one K=128 matmul gives S^T = q.k + 8*(bias_t + bias_p + mask).
     The onehot/bias halves are DMA'd once and fanned out on Pool; the
     late fanouts drain into phase 2 where DMA/Pool have slack.
  3. v natural per (b): vA[128, jc, h, 65] with a ones column at index 64,
     so PV also produces the softmax row-sum. Batch-1's v-projection runs
     inside batch-0's attention loop (fills PE gaps while Act runs exp).
  4. S^T computed per j-chunk into two packed [128,1536] PSUM tiles
     (chunks 0,2,4 | 1,3,5), one big exp each (scale=1/8) -> P^T tiles.
  5. PV accumulates y^T [65, i] with region-aware start/stop flags over the
     triangular chunk structure (tile-A chunks first so the P^T-A buffer
     frees early). Row 64 = rowsum; DVE reciprocal -> Pool broadcast ->
     DVE multiply writes normalized Y^T.
  6. out = Y^T-contract with Wo -> [rows, 512] natural, DMA out. Batch-0
     finals hide inside batch-1's attention; batch-1 finals use the (by
     then idle) S-psum slots so they pipeline without waiting on copies.
"""

import sys

sys.path.insert(0, "/opt/trn_rl_repo")

import numpy as np

import concourse.bacc as bacc
import concourse.bass as bass
import concourse.mybir as mybir
import concourse.tile as tile
from concourse import bass_utils

F32 = mybir.dt.float32
F32R = mybir.dt.float32r
BF16 = mybir.dt.bfloat16
EXP = mybir.ActivationFunctionType.Exp

B_TOT = 16      # total batches
H = 8           # heads
T = 48          # timesteps
NP = 16         # particles
C = 512         # channels
HD = C // H     # 64 head dim
L = NP * T      # 768 sequence per batch
NCORES = 8
BPC = B_TOT // NCORES   # 2 batches per core
R = BPC * L             # 1536 rows per core
MASK = -1600.0          # pre-scale mask add: -200 * sqrt(hd)
SCALE = 0.125           # 1/sqrt(hd)

# S^T tile packing: (jc, base_col, i_lo); i_hi is always L. Tile A holds
# S^T packing: three 2-bank [128,1024] psum tiles per head, each one exp.
# Entries are (jc, base_col, i_lo); i_hi is always L. All psum pieces are
# >=256 wide and bank-contained (banks at cols 0/512). Three tiles per head
# over a 2-buffer pool keeps the slot rotation aligned with the exp that
# frees it (no slow consumers in the rotation).
S_TILES = (((0, 0, 0), (4, 768, 512)),      # jc0 + jc4
           ((1, 0, 0), (5, 768, 512)),      # jc1 + jc5
           ((2, 0, 256), (3, 512, 256)))    # jc2 + jc3
QK_PIECES = {0: ((0, 512), (512, 768)),
             256: ((256, 512), (512, 768)),
             512: ((512, 768),)}
# PV write plan: (jc, i0, i1, start, stop). Regions of yp: [0,256) written
# by jc0,1; [256,512) by jc0..3; [512,768) by all six. Order 0,4,2,3,1,5
# consumes pt tiles in exp-completion order (T1, T2, T3) so PV never
# stalls mid-flight on a late exp, and early pt slots free for reuse.
PV_PLAN = (
    (0, 0, 512, True, False), (0, 512, 768, True, False),
    (4, 512, 768, False, False),
    (1, 0, 256, False, True), (1, 256, 512, False, False),
    (1, 512, 768, False, False),
    (5, 512, 768, False, False),
    (2, 256, 512, False, False), (2, 512, 768, False, False),
    (3, 256, 512, False, True), (3, 512, 768, False, True),
)
S_BASE = {jc: (ti, base, i_lo)
          for ti, tiledef in enumerate(S_TILES)
          for jc, base, i_lo in tiledef}


def build_nc():
    nc = bacc.Bacc("TRN2", target_bir_lowering=False, debug=False)

    xt_d = nc.dram_tensor("xt", [BPC, 4, 128, L], BF16, kind="ExternalInput").ap()
    wq_d = nc.dram_tensor("wq", [4, 128, C], BF16, kind="ExternalInput").ap()
    wk_d = nc.dram_tensor("wk", [4, 128, C], BF16, kind="ExternalInput").ap()
    wv_d = nc.dram_tensor("wv", [4, 128, C], BF16, kind="ExternalInput").ap()
    wo_d = nc.dram_tensor("wo", [4, 128, C], F32R, kind="ExternalInput").ap()
    oh_d = nc.dram_tensor("onehot", [64, L], BF16, kind="ExternalInput").ap()
    kb_d = nc.dram_tensor("kbias", [H, 64, L], BF16, kind="ExternalInput").ap()
    out_d = nc.dram_tensor("out", [R, C], F32, kind="ExternalOutput").ap()

    with tile.TileContext(nc) as tc:
        _body(tc, xt_d, wq_d, wk_d, wv_d, wo_d, oh_d, kb_d, out_d)
    nc.compile()
    return nc


def _body(tc, xt_d, wq_d, wk_d, wv_d, wo_d, oh_d, kb_d, out_d):
    nc = tc.nc
    from contextlib import ExitStack

    with ExitStack() as ctx:
        const = ctx.enter_context(tc.tile_pool(name="const", bufs=1))
        persist = ctx.enter_context(tc.tile_pool(name="persist", bufs=1))
        # wv + xT[1] live into phase 2: batch-1's v-projection runs there
        wv_pool = ctx.enter_context(tc.tile_pool(name="wv_sb", bufs=1))
        xt1_pool = ctx.enter_context(tc.tile_pool(name="xt1_sb", bufs=1))

        ones_f32 = const.tile([128, 64], F32, name="ones_f32")
        nc.vector.memset(ones_f32, 1.0)
        zbias = const.tile([128, 1], F32, name="zbias")
        nc.vector.memset(zbias, 0.0)
        wo_sb = const.tile([128, 4, C], F32R, name="wo_sb")  # DMA'd in phase 1

        # persistent per-core tensors: augmented q/k per (b,h), v, y^T
        qa = {(b, h): persist.tile([128, L], BF16, name=f"qa{b}_{h}",
                                   tag=f"qa{b}_{h}")
              for b in range(BPC) for h in range(H)}
        ka = {(b, h): persist.tile([128, L], BF16, name=f"ka{b}_{h}",
                                   tag=f"ka{b}_{h}")
              for b in range(BPC) for h in range(H)}
        vA = [persist.tile([128, 6, H, HD + 1], F32R, name=f"vA{b}", tag=f"vA{b}")
              for b in range(BPC)]
        # per-(batch, cc) so batch-0 finals don't falsely serialize against
        # batch-1's normalize writes (tile-granular dependency tracking)
        yT = {(b, c): persist.tile([128, L], F32R, name=f"yT{b}_{c}",
                                   tag=f"yT{b}_{c}")
              for b in range(BPC) for c in range(4)}
        for b in range(BPC):
            nc.vector.tensor_copy(
                out=vA[b][:, :, :, HD:HD + 1],
                in_=ones_f32[:, 0:48].rearrange("p (a h) -> p a h", a=6).unsqueeze(3))

        wv_sb = wv_pool.tile([128, 4, C], BF16, name="wv_sb")
        xT1 = xt1_pool.tile([128, 4, L], BF16, name="xT1", tag="xT1")

        def proj_v(b, xT_b, ps):
            def f(l):
                for ci in range(4):
                    nc.tensor.matmul(
                        ps[:, 0:C], lhsT=xT_b[:, ci, l * 128:(l + 1) * 128],
                        rhs=wv_sb[:, ci, :],
                        start=(ci == 0), stop=(ci == 3))
                nc.vector.tensor_copy(
                    out=vA[b][:, l, :, 0:HD],
                    in_=ps[:, 0:C].rearrange("p (h d) -> p h d", h=H))
            return f

        # ---------------- phase 1: DMAs + projections into aug tiles ----------------
        with tc.tile_pool(name="wqk", bufs=1) as wqk_pool, \
             tc.tile_pool(name="xt0_sb", bufs=1) as xt0_pool, \
             tc.tile_pool(name="pj_ps", bufs=4, space="PSUM") as pj_psum, \
             tc.tile_pool(name="gap_ps", bufs=1, space="PSUM") as gap_psum, \
             tc.tile_pool(name="vj_ps", bufs=2, space="PSUM") as vj_psum:

            # parked 2-bank tile: keeps banks 4-5 untouched in phase 1 so
            # the first attention S-tile allocates with zero wait
            gap_psum.tile([128, 1024], F32, name="gapt", tag="gapt")
            xT0 = xt0_pool.tile([128, 4, L], BF16, name="xT0", tag="xT0")
            wq_sb = wqk_pool.tile([128, 4, C], BF16, name="wq_sb")
            wk_sb = wqk_pool.tile([128, 4, C], BF16, name="wk_sb")

            # critical-path DMAs first, in consumption order (q projections
            # of batch 0 run first, so xt0+wq lead); the constant
            # (onehot/bias) DMAs queue behind and drain into phase 2
            for ci in range(4):
                nc.sync.dma_start(out=xT0[:, ci, :], in_=xt_d[0, ci])
                nc.sync.dma_start(out=wq_sb[:, ci, :], in_=wq_d[ci])
            for ci in range(4):
                nc.sync.dma_start(out=wk_sb[:, ci, :], in_=wk_d[ci])
            for ci in range(4):
                nc.sync.dma_start(out=wv_sb[:, ci, :], in_=wv_d[ci])
            for ci in range(4):
                nc.sync.dma_start(out=xT1[:, ci, :], in_=xt_d[1, ci])
            nc.sync.dma_start(out=qa[(0, 0)][64:128, :], in_=oh_d)
            for h in range(H):
                nc.sync.dma_start(out=ka[(0, h)][64:128, :], in_=kb_d[h])
            nc.sync.dma_start(out=wo_sb, in_=wo_d.rearrange("c p o -> p c o"))
            # onehot/bias fanout for batch 0 (batch 1's fans out in phase 2)
            for h in range(1, H):
                nc.gpsimd.tensor_copy(out=qa[(0, h)][64:128, :],
                                      in_=qa[(0, 0)][64:128, :])

            def proj_qk(b, cc, w_sb, xT_b, dst):
                # two single-bank [128,384] psums per projection; c_out 0:64
                # -> head 2cc, 64:128 -> 2cc+1. One-bank tiles keep two PSUM
                # banks free in phase 1 so the first S-tile of the attention
                # phase can allocate before the last projections drain.
                for off in (0, 384):
                    ps = pj_psum.tile([128, 384], F32, name="pjp", tag="pjp")
                    for ci in range(4):
                        nc.tensor.matmul(
                            ps,
                            lhsT=w_sb[:, ci, cc * 128:(cc + 1) * 128],
                            rhs=xT_b[:, ci, off:off + 384],
                            start=(ci == 0), stop=(ci == 3))
                    nc.scalar.copy(out=dst[(b, 2 * cc)][0:64, off:off + 384],
                                   in_=ps[0:64, :])
                    nc.vector.tensor_copy(
                        out=dst[(b, 2 * cc + 1)][0:64, off:off + 384],
                        in_=ps[64:128, :])

            # projections in DMA-arrival order: q(b0), k(b0), q(b1), k(b1),
            # then v(b0); batch-1's v-projection runs inside batch-0's
            # attention loop
            for cc in range(4):
                proj_qk(0, cc, wq_sb, xT0, qa)
            for cc in range(4):
                proj_qk(0, cc, wk_sb, xT0, ka)
            for l in range(6):
                proj_v(0, xT0, vj_psum.tile([128, C], F32, name="vjp", tag="vjp"))(l)
            for cc in range(4):
                proj_qk(1, cc, wq_sb, xT1, qa)
            for cc in range(4):
                proj_qk(1, cc, wk_sb, xT1, ka)

        # -------- phase 2: attention, software-pipelined one head deep --------
        # Per iteration the PE stream is QK(h), PV(h-1), <fill>: the exps of
        # head h run on Act while the PE does PV of h-1, so neither engine
        # waits on the other's serial chain. Fill work: batch-1 v-projection
        # during batch-0's loop, batch-0 output chunks during batch-1's; the
        # fills use the second yp buffer so the S-slot rotation stays clean.
        with tc.tile_pool(name="p_sb", bufs=4) as p_pool, \
             tc.tile_pool(name="ys_sb", bufs=3) as ys_pool, \
             tc.tile_pool(name="rc_sb", bufs=2) as rc_pool, \
             tc.tile_pool(name="bc_sb", bufs=2) as bc_pool, \
             tc.tile_pool(name="fo_sb", bufs=3) as fo_pool, \
             tc.tile_pool(name="y_ps", bufs=2, space="PSUM") as y_psum, \
             tc.tile_pool(name="s_ps", bufs=2, space="PSUM") as s_psum:

            def attn_qk(b, h):
                qa_t, ka_t = qa[(b, h)], ka[(b, h)]
                pts = []
                for tiledef in S_TILES:
                    sp = s_psum.tile([128, 1024], F32, name="sp", tag="sp")
                    for jc, base, i_lo in tiledef:
                        for p0, p1 in QK_PIECES[i_lo]:
                            nc.tensor.matmul(
                                sp[:, base + p0 - i_lo:base + p1 - i_lo],
                                lhsT=ka_t[:, jc * 128:(jc + 1) * 128],
                                rhs=qa_t[:, p0:p1],
                                start=True, stop=True)
                    pt = p_pool.tile([128, 1024], F32R, name="pt", tag="pt")
                    nc.scalar.activation(out=pt, in_=sp, func=EXP,
                                         bias=zbias, scale=SCALE)
                    pts.append(pt)
                return pts

            def attn_pv(b, h, pts, last=False):
                # PV: y^T [65, i]; row 64 = rowsum (ones col). Region-aware
                # start/stop over the triangular chunk structure.
                yp = y_psum.tile([128, L], F32, name="yp", tag="yp")
                for jc, i0, i1, st, sp_ in PV_PLAN:
                    ti, base, i_lo = S_BASE[jc]
                    nc.tensor.matmul(
                        yp[0:HD + 1, i0:i1],
                        lhsT=vA[b][:, jc, h, :],
                        rhs=pts[ti][:, base + i0 - i_lo:base + i1 - i_lo],
                        start=st, stop=sp_)

                # normalize: reciprocal of the rowsum lands on a partition-0
                # tile (partition_broadcast reads PHYSICAL partition 0); the
                # broadcast and multiply run on Pool so the DVE only does the
                # slot-releasing ys copy + reciprocal (keeps yp cycling fast).
                # The last head's multiply runs on the (by then idle) DVE -
                # it gates the closing output chunks, so latency matters.
                cc, par = divmod(h, 2)
                rcp = rc_pool.tile([1, L], F32, name="rcp", tag="rcp")
                ys = ys_pool.tile([HD + 1, L], F32, name="ys", tag="ys")
                nc.vector.tensor_copy(out=ys, in_=yp[0:HD + 1, :])
                nc.vector.reciprocal(out=rcp, in_=ys[HD:HD + 1, :])
                if last:
                    # PE broadcast (K=1 matmul) + DVE multiply: ~1us shorter
                    # chain than Pool, and both engines are idle at the tail
                    bcp = s_psum.tile([64, L], F32, name="bcp", tag="sp")
                    for p0, p1 in ((0, 512), (512, L)):
                        nc.tensor.matmul(bcp[:, p0:p1],
                                         lhsT=ones_f32[0:1, 0:HD],
                                         rhs=rcp[:, p0:p1],
                                         start=True, stop=True)
                    nc.vector.tensor_tensor(
                        yT[(b, cc)][par * 64:par * 64 + 64, :],
                        ys[0:HD, :], bcp, mybir.AluOpType.mult)
                else:
                    bcs = bc_pool.tile([64, L], F32, name="bcs", tag="bcs")
                    nc.gpsimd.partition_broadcast(out_ap=bcs, in_ap=rcp)
                    nc.gpsimd.tensor_tensor(
                        yT[(b, cc)][par * 64:par * 64 + 64, :],
                        ys[0:HD, :], bcs, mybir.AluOpType.mult)

            def final(b, ic):
                # output projection chunk; alternates with yp in the 2-buf
                # y-psum pool so PV never waits on the output copies
                fp = y_psum.tile([128, C], F32, name="fp", tag="yp")
                for c4 in range(4):
                    nc.tensor.matmul(
                        fp[:, 0:C],
                        lhsT=yT[(b, c4)][:, ic * 128:(ic + 1) * 128],
                        rhs=wo_sb[:, c4, :],
                        start=(c4 == 0), stop=(c4 == 3))
                fo = fo_pool.tile([128, C], F32, name="fo", tag="fo")
                nc.vector.tensor_copy(out=fo, in_=fp[:, 0:C])
                nc.sync.dma_start(
                    out=out_d[b * L + ic * 128:b * L + (ic + 1) * 128, :],
                    in_=fo)

            prev = None
            for h in range(H):
                cur = attn_qk(0, h)
                if prev is not None:
                    attn_pv(0, h - 1, prev)
                    if h - 1 < 6:
                        proj_v(1, xT1, y_psum.tile([128, L], F32,
                                                   name="vjp", tag="yp"))(h - 1)
                    # deferred constant fanouts for batch 1 ride the DMA
                    # queue (idle in phase 2) - engines stay untouched
                    nc.sync.dma_start(out=qa[(1, h - 1)][64:128, :],
                                      in_=qa[(0, 0)][64:128, :])
                    nc.sync.dma_start(out=ka[(1, h - 1)][64:128, :],
                                      in_=ka[(0, h - 1)][64:128, :])
                prev = cur
            attn_pv(0, H - 1, prev)
            nc.sync.dma_start(out=qa[(1, H - 1)][64:128, :],
                              in_=qa[(0, 0)][64:128, :])
            nc.sync.dma_start(out=ka[(1, H - 1)][64:128, :],
                              in_=ka[(0, H - 1)][64:128, :])

            prev = None
            for h in range(H):
                cur = attn_qk(1, h)
                if prev is not None:
                    attn_pv(1, h - 1, prev)
                    if h >= 2:
                        final(0, h - 2)  # hide under batch-1 attention
                prev = cur
            attn_pv(1, H - 1, prev, last=True)
            # tail finals: accumulate the c4=0..2 thirds (ready - they only
            # need heads 0..5) while the last head's normalize chain runs,
            # then close each with the c4=3 third + copy-out
            fps = []
            for ic in range(4):
                fp = y_psum.tile([128, C], F32, name="fp", tag="yp") \
                    if ic % 2 == 0 else \
                    s_psum.tile([128, C], F32, name="fp", tag="sp")
                for c4 in range(3):
                    nc.tensor.matmul(
                        fp[:, 0:C],
                        lhsT=yT[(1, c4)][:, ic * 128:(ic + 1) * 128],
                        rhs=wo_sb[:, c4, :],
                        start=(c4 == 0), stop=False)
                fps.append(fp)
            for ic in range(6):
                if ic < 4:
                    fp = fps[ic]
                    nc.tensor.matmul(
                        fp[:, 0:C],
                        lhsT=yT[(1, 3)][:, ic * 128:(ic + 1) * 128],
                        rhs=wo_sb[:, 3, :],
                        start=False, stop=True)
                    fo = fo_pool.tile([128, C], F32, name="fo", tag="fo")
                    if ic % 2 == 0:
                        nc.scalar.copy(out=fo, in_=fp[:, 0:C])
                    else:
                        nc.vector.tensor_copy(out=fo, in_=fp[:, 0:C])
                    eng = nc.sync if ic % 2 == 0 else nc.gpsimd
                    eng.dma_start(
                        out=out_d[L + ic * 128:L + (ic + 1) * 128, :], in_=fo)
                else:
                    final(1, ic)


def host_tables(rel_pos_bias, particle_rel_pos_bias):
    """onehot [64, L] and kbias [H, 64, L] fp32 host constants (time-major)."""
    rel_pos_bias = np.asarray(rel_pos_bias, np.float32)        # [2T-1, H]
    particle_rel_pos_bias = np.asarray(particle_rel_pos_bias, np.float32)  # [2NP-1, H]
    idx = np.arange(L)
    it, ip = idx // NP, idx % NP          # t1(i), n1(i)  (time-major rows)
    onehot = np.zeros((64, L), np.float32)
    onehot[it, idx] = 1.0
    onehot[T + ip, idx] = 1.0

    jt, jn = idx // NP, idx % NP          # t2(j), n2(j)
    t1 = np.arange(T)[:, None]
    bt = rel_pos_bias[(jt[None, :] - t1) + (T - 1)]            # [T, L, H]
    ktop = 8.0 * np.transpose(bt, (2, 0, 1))                   # [H, T, L]
    ktop = ktop + np.where(jt[None, :] > t1, MASK, 0.0)[None]
    n1 = np.arange(NP)[:, None]
    bp = particle_rel_pos_bias[(jn[None, :] - n1) + (NP - 1)]  # [NP, L, H]
    kbot = 8.0 * np.transpose(bp, (2, 0, 1))                   # [H, NP, L]
    kbias = np.concatenate([ktop, kbot], axis=1).astype(np.float32)
    return onehot, np.ascontiguousarray(kbias)


def make_in_maps(x, Wq, Wk, Wv, Wo, rel_pos_bias, particle_rel_pos_bias):
    import ml_dtypes
    bf16 = ml_dtypes.bfloat16
    x = np.ascontiguousarray(np.asarray(x, np.float32))
    ws = [np.ascontiguousarray(np.asarray(w, np.float32).reshape(4, 128, C))
          for w in (Wq, Wk, Wv, Wo)]
    onehot, kbias = host_tables(rel_pos_bias, particle_rel_pos_bias)
    # time-major rows r = t*NP + n, then pre-transposed to [C, rows] chunks
    xs = x.reshape(B_TOT, NP, T, C).transpose(0, 2, 1, 3)      # (B, T, NP, C)
    in_maps = []
    for c in range(NCORES):
        xc = xs[BPC * c:BPC * (c + 1)].reshape(BPC, L, C)
        xt = np.ascontiguousarray(
            xc.transpose(0, 2, 1).reshape(BPC, 4, 128, L))     # (b, ci, p, i)
        in_maps.append({
            "xt": xt.astype(bf16), "wq": ws[0].astype(bf16),
            "wk": ws[1].astype(bf16), "wv": ws[2].astype(bf16), "wo": ws[3],
            "onehot": onehot.astype(bf16), "kbias": kbias.astype(bf16),
        })
    return in_maps


def unshard_core(out_core):
    """[R, C] time-major rows -> (BPC, NP, T, C)."""
    return out_core.reshape(BPC, T, NP, C).transpose(0, 2, 1, 3)


_NC_CACHE = None


def _get_nc():
    global _NC_CACHE
    if _NC_CACHE is None:
        _NC_CACHE = build_nc()
    return _NC_CACHE


def kernel(x, Wq, Wk, Wv, Wo, rel_pos_bias, particle_rel_pos_bias):
    in_maps = make_in_maps(x, Wq, Wk, Wv, Wo, rel_pos_bias, particle_rel_pos_bias)
    res = bass_utils.run_bass_kernel_spmd(
        _get_nc(), in_maps, core_ids=list(range(NCORES)))
    outs = [unshard_core(res.results[c]["out"]) for c in range(NCORES)]
    return np.ascontiguousarray(np.concatenate(outs, axis=0))
